# revision 15
# baseline (speedup 1.0000x reference)
import sys

sys.path.insert(0, "/opt/trn_rl_repo")

import numpy as np
import ml_dtypes
from contextlib import ExitStack

import concourse.bass as bass
import concourse.bacc as bacc
import concourse.mybir as mybir
import concourse.tile as tile
from concourse.bass_utils import run_bass_kernel_spmd

B, S, D, MD = 4, 4096, 1024, 512
NCORES = 8
RPC = B * S // NCORES      # rows (tokens) per core = 2048
TT = 512                   # tokens per tile
NT = RPC // TT             # 4 tiles per core
P = 128
DB = D // P                # 8 k-blocks for D
MB = MD // P               # 4 blocks for MD
F32 = mybir.dt.float32
BF16 = mybir.dt.bfloat16
AF = mybir.ActivationFunctionType
OP = mybir.AluOpType
BF = ml_dtypes.bfloat16

_cache = {}


def _build_nc():
    nc = bacc.Bacc("TRN2", target_bir_lowering=False, debug=False,
                   num_devices=NCORES)

    xT = nc.dram_tensor("xT", [D, RPC], F32, kind="ExternalInput")
    wd = nc.dram_tensor("wd", [D, MD], BF16, kind="ExternalInput")
    wq = nc.dram_tensor("wq", [MD, MD], BF16, kind="ExternalInput")
    wk = nc.dram_tensor("wk", [MD, MD], BF16, kind="ExternalInput")
    wv = nc.dram_tensor("wv", [MD, MD], BF16, kind="ExternalInput")
    w0q = nc.dram_tensor("w0q", [MD, MD], BF16, kind="ExternalInput")
    w0k = nc.dram_tensor("w0k", [MD, MD], BF16, kind="ExternalInput")
    w1 = nc.dram_tensor("w1", [MD, MD], BF16, kind="ExternalInput")
    wu = nc.dram_tensor("wu", [MD, D], BF16, kind="ExternalInput")
    # per-partition biases [128, MB] fp32 (applied via ACT Identity)
    bd_i = nc.dram_tensor("bd_i", [P, MB], F32, kind="ExternalInput")
    bq_i = nc.dram_tensor("bq_i", [P, MB], F32, kind="ExternalInput")
    bk_i = nc.dram_tensor("bk_i", [P, MB], F32, kind="ExternalInput")
    bv_i = nc.dram_tensor("bv_i", [P, MB], F32, kind="ExternalInput")
    bu_row = nc.dram_tensor("bu_row", [1, D], BF16, kind="ExternalInput")
    # gelu biases stay per-partition (free via ACT)
    c0q_i = nc.dram_tensor("c0q_i", [P, MB], F32, kind="ExternalInput")
    c0k_i = nc.dram_tensor("c0k_i", [P, MB], F32, kind="ExternalInput")
    g_tile_i = nc.dram_tensor("g_tile_i", [P, RPC], BF16, kind="ExternalInput")
    lr_i = nc.dram_tensor("lr_i", [P, 1], F32, kind="ExternalInput")

    y = nc.dram_tensor("y", [RPC, D], F32, kind="ExternalOutput")
    carry = nc.dram_tensor("carry", [P, MB], BF16, kind="ExternalOutput")

    with ExitStack() as ctx:
        tc = ctx.enter_context(tile.TileContext(nc))
        wpool = ctx.enter_context(tc.tile_pool(name="wpool", bufs=1))
        persist = ctx.enter_context(tc.tile_pool(name="persist", bufs=1))
        work = ctx.enter_context(tc.tile_pool(name="work", bufs=2))
        stats = ctx.enter_context(tc.tile_pool(name="stats", bufs=3))
        psum = ctx.enter_context(tc.tile_pool(name="psum", bufs=4,
                                              space="PSUM"))
        psum2 = ctx.enter_context(tc.tile_pool(name="psum2", bufs=2,
                                               space="PSUM"))
        outp = ctx.enter_context(tc.tile_pool(name="outp", bufs=3))

        # ---- load weights / constants (once) ----
        wd_sb = wpool.tile([P, DB, MD], BF16)
        wd_r = wd.rearrange("(ko ki) m -> ki ko m", ki=P)
        for kb in range(DB):
            nc.sync.dma_start(wd_sb[:, kb, :], wd_r[:, kb, :])
        w_sbs = {}
        for name, t in (("wq", wq), ("wk", wk), ("wv", wv), ("w0q", w0q),
                        ("w0k", w0k), ("w1", w1)):
            sb = wpool.tile([P, MB, MD], BF16, tag=name)
            t_r = t.rearrange("(ko ki) m -> ki ko m", ki=P)
            for kb in range(MB):
                nc.sync.dma_start(sb[:, kb, :], t_r[:, kb, :])
            w_sbs[name] = sb
        wu_sb = wpool.tile([P, MB, D], BF16)
        wu_r = wu.rearrange("(ko ki) m -> ki ko m", ki=P)
        for kb in range(MB):
            nc.sync.dma_start(wu_sb[:, kb, :], wu_r[:, kb, :])

        bias_sbs = {}
        for name, t in (("bd", bd_i), ("bq", bq_i), ("bk", bk_i),
                        ("bv", bv_i)):
            sb = wpool.tile([P, MB], F32, tag="b" + name)
            nc.sync.dma_start(sb, t[:])
            bias_sbs[name] = sb
        bu_sb = wpool.tile([1, D], BF16)
        nc.sync.dma_start(bu_sb, bu_row[:])
        c0_sbs = {}
        for name, t in (("c0q", c0q_i), ("c0k", c0k_i)):
            sb = wpool.tile([P, MB], F32, tag=name)
            nc.sync.dma_start(sb, t[:])
            c0_sbs[name] = sb
        g_tile = persist.tile([P, RPC], BF16)
        nc.sync.dma_start(g_tile, g_tile_i[:])
        lr_sb = wpool.tile([P, 1], F32)
        nc.sync.dma_start(lr_sb, lr_i[:])

        ones_mean = wpool.tile([P, P], BF16)
        nc.vector.memset(ones_mean, 1.0 / MD)
        ones_one = wpool.tile([P, P], BF16)
        nc.vector.memset(ones_one, 1.0)
        ones_row = wpool.tile([1, P], BF16)
        nc.vector.memset(ones_row, 1.0)
        eps_sb = wpool.tile([P, 1], F32)
        nc.vector.memset(eps_sb, 1e-5)

        scan_b = persist.tile([P, MB, RPC], BF16)

        def proj(h_bf, w_sb, bias_sb, tag, kblocks=MB):
            o = work.tile([P, MB, TT], BF16, tag=tag)
            for mb in range(MB):
                ps = psum.tile([P, TT], F32, tag="ps")
                for kb in range(kblocks):
                    nc.tensor.matmul(ps, w_sb[:, kb, mb * P:(mb + 1) * P],
                                     h_bf[:, kb, :], start=(kb == 0),
                                     stop=(kb == kblocks - 1))
                nc.scalar.activation(o[:, mb, :], ps, AF.Identity,
                                     bias=bias_sb[:, mb:mb + 1])
            return o

        def lnorm(pre, tag):
            sq = work.tile([P, MB, TT], BF16, tag="lnsq")
            nc.vector.tensor_mul(sq, pre, pre)
            m_ps = psum.tile([P, TT], F32, tag="ps")
            for kb in range(MB):
                nc.tensor.matmul(m_ps, ones_mean, pre[:, kb, :],
                                 start=(kb == 0), stop=(kb == MB - 1))
            e2_ps = psum.tile([P, TT], F32, tag="ps")
            for kb in range(MB):
                nc.tensor.matmul(e2_ps, ones_mean, sq[:, kb, :],
                                 start=(kb == 0), stop=(kb == MB - 1))
            m_sb = stats.tile([P, TT], BF16, tag="m")
            nc.vector.tensor_copy(m_sb, m_ps)
            msq = stats.tile([P, TT], BF16, tag="msq")
            nc.vector.tensor_mul(msq, m_sb, m_sb)
            var = stats.tile([P, TT], BF16, tag="var")
            nc.vector.tensor_tensor(var, e2_ps, msq, OP.subtract)
            std = stats.tile([P, TT], F32, tag="std")
            nc.scalar.activation(std, var, AF.Sqrt, bias=eps_sb)
            rstd_f = stats.tile([P, TT], F32, tag="rstdf")
            nc.vector.reciprocal_approx_fast(rstd_f, std)
            rstd = stats.tile([P, TT], BF16, tag="rstd")
            nc.vector.tensor_copy(rstd, rstd_f)
            mr = stats.tile([P, TT], BF16, tag="mr")
            nc.vector.tensor_mul(mr, m_sb, rstd)
            out_n = work.tile([P, MB, TT], BF16, tag="lnn")
            for kb in range(MB):
                nc.vector.tensor_mul(out_n[:, kb, :], pre[:, kb, :], rstd)
                nc.vector.tensor_tensor(out_n[:, kb, :], out_n[:, kb, :], mr,
                                        OP.subtract)
            return out_n

        def memmlp1(a_bf, w0name, c0name):
            g = work.tile([P, MB, TT], BF16, tag="gg")
            w0_sb = w_sbs[w0name]
            for mb in range(MB):
                ps = psum.tile([P, TT], F32, tag="ps")
                for kb in range(MB):
                    nc.tensor.matmul(ps, w0_sb[:, kb, mb * P:(mb + 1) * P],
                                     a_bf[:, kb, :], start=(kb == 0),
                                     stop=(kb == MB - 1))
                nc.scalar.activation(g[:, mb, :], ps, AF.Gelu_apprx_tanh,
                                     bias=c0_sbs[c0name][:, mb:mb + 1])
            return g

        def emit_out(t0, rs, off=0, ln=TT):
            for tb in range(ln // P):
                pso = psum2.tile([P, D], F32, tag="pso")
                for nh in range(2):
                    sl = slice(nh * 512, (nh + 1) * 512)
                    for kb in range(MB):
                        nc.tensor.matmul(
                            pso[:, sl],
                            rs[:, kb, off + tb * P:off + (tb + 1) * P],
                            wu_sb[:, kb, sl], start=(kb == 0), stop=False)
                    nc.tensor.matmul(pso[:, sl], ones_row, bu_sb[:, sl],
                                     start=False, stop=True)
                o_sb = outp.tile([P, D], F32, tag="osb")
                nc.scalar.activation(o_sb, pso, AF.Identity)
                nc.sync.dma_start(y[t0 + tb * P:t0 + (tb + 1) * P, :], o_sb)

        pending = None
        for ti in range(NT):
            t0 = ti * TT
            # load x with DMA-cast fp32->bf16 (software DGE casts in flight)
            xbf = work.tile([P, DB, TT], BF16, tag="xbf")
            for kb in range(DB):
                nc.gpsimd.dma_start(xbf[:, kb, :],
                                    xT[kb * P:(kb + 1) * P, t0:t0 + TT])
            # h = x@Wd + bd   (feature-major [MD, TT])
            h_bf = work.tile([P, MB, TT], BF16, tag="h")
            for mb in range(MB):
                ps = psum.tile([P, TT], F32, tag="ps")
                for kb in range(DB):
                    nc.tensor.matmul(ps, wd_sb[:, kb, mb * P:(mb + 1) * P],
                                     xbf[:, kb, :], start=(kb == 0),
                                     stop=(kb == DB - 1))
                nc.scalar.activation(h_bf[:, mb, :], ps, AF.Identity,
                                     bias=bias_sbs["bd"][:, mb:mb + 1])
            qpre = proj(h_bf, w_sbs["wq"], bias_sbs["bq"], "pre")
            kpre = proj(h_bf, w_sbs["wk"], bias_sbs["bk"], "pre")
            v_bf = proj(h_bf, w_sbs["wv"], bias_sbs["bv"], "vbf")
            qn = lnorm(qpre, "q")
            kn = lnorm(kpre, "k")
            if pending is not None:
                emit_out(*pending)
                pending = None
            gq = memmlp1(qn, "w0q", "c0q")
            gk = memmlp1(kn, "w0k", "c0k")
            # retrieved = gq @ W1
            retr = work.tile([P, MB, TT], BF16, tag="retr")
            w1_sb = w_sbs["w1"]
            for mb in range(MB):
                ps = psum.tile([P, TT], F32, tag="ps")
                for kb in range(MB):
                    nc.tensor.matmul(ps, w1_sb[:, kb, mb * P:(mb + 1) * P],
                                     gq[:, kb, :], start=(kb == 0),
                                     stop=(kb == MB - 1))
                nc.scalar.activation(retr[:, mb, :], ps, AF.Identity)
            # pred = gk @ W1 ; d = pred - v, then d^2 in place
            d_bf = work.tile([P, MB, TT], BF16, tag="d")
            for mb in range(MB):
                ps = psum.tile([P, TT], F32, tag="ps")
                for kb in range(MB):
                    nc.tensor.matmul(ps, w1_sb[:, kb, mb * P:(mb + 1) * P],
                                     gk[:, kb, :], start=(kb == 0),
                                     stop=(kb == MB - 1))
                nc.vector.tensor_tensor(d_bf[:, mb, :], ps, v_bf[:, mb, :],
                                        OP.subtract)
            nc.vector.tensor_mul(d_bf, d_bf, d_bf)
            ssum_ps = psum.tile([P, TT], F32, tag="ps")
            for kb in range(MB):
                nc.tensor.matmul(ssum_ps, ones_one, d_bf[:, kb, :],
                                 start=(kb == 0), stop=(kb == MB - 1))
            ssc = stats.tile([P, TT], BF16, tag="ssc")
            nc.vector.tensor_scalar_mul(ssc, ssum_ps, lr_sb)
            inp = work.tile([P, MB, TT], BF16, tag="inp")
            for mb in range(MB):
                nc.vector.tensor_mul(inp[:, mb, :], v_bf[:, mb, :], ssc)
            # chained scan along tokens; split the last tile's tail so the
            # final out-phase starts as soon as the first half has scanned
            halves = ((0, TT),) if ti < NT - 1 else ((0, TT // 2),
                                                     (TT // 2, TT))
            for (a, b) in halves:
                for mb in range(MB):
                    init = 0.0 if ti == 0 and a == 0 else                         scan_b[:, mb, t0 + a - 1:t0 + a]
                    nc.vector.tensor_tensor_scan(
                        scan_b[:, mb, t0 + a:t0 + b],
                        g_tile[:, t0 + a:t0 + b],
                        inp[:, mb, a:b], init, OP.mult, OP.add)
                nc.vector.tensor_tensor(retr[:, :, a:b], retr[:, :, a:b],
                                        scan_b[:, :, t0 + a:t0 + b], OP.add)
                if ti == NT - 1:
                    emit_out(t0 + a, retr, a, b - a)
            if ti < NT - 1:
                pending = (t0, retr)
        nc.sync.dma_start(carry[:], scan_b[:, :, RPC - 1])
    nc.compile()
    return nc


def _prep_shared(inputs):
    bf = lambda a: np.ascontiguousarray(a).astype(BF)
    f32 = lambda a: np.ascontiguousarray(a, dtype=np.float32)
    W0 = inputs["W0"].astype(np.float32)
    g_val = 1.0 - 1.0 / (1.0 + np.exp(-float(inputs["forget_factor"][0])))
    g_bf = float(np.float32(g_val).astype(BF))
    lr = float(inputs["adaptive_lr"][0])
    per_part = lambda b: f32(b.reshape(MB, P).T)  # [512] -> [128, MB]
    shared = {
        "wd": bf(inputs["Wd"]), "wq": bf(inputs["Wq"]), "wk": bf(inputs["Wk"]),
        "wv": bf(inputs["Wv"]),
        "w0q": bf(inputs["q_gamma"][:, None] * W0),
        "w0k": bf(inputs["k_gamma"][:, None] * W0),
        "w1": bf(inputs["W1"]), "wu": bf(inputs["Wu"]),
        "bd_i": per_part(inputs["bd"]), "bq_i": per_part(inputs["bq"]),
        "bk_i": per_part(inputs["bk"]), "bv_i": per_part(inputs["bv"]),
        "bu_row": bf(inputs["bu"][None, :]),
        "c0q_i": per_part(inputs["q_beta"].astype(np.float32) @ W0),
        "c0k_i": per_part(inputs["k_beta"].astype(np.float32) @ W0),
        "g_tile_i": np.full((P, RPC), g_bf, dtype=BF),
        "lr_i": np.full((P, 1), lr / MD, dtype=np.float32),
    }
    return shared, g_bf


def kernel(**inputs):
    if "nc" not in _cache:
        _cache["nc"] = _build_nc()
    nc = _cache["nc"]
    shared, g_bf = _prep_shared(inputs)
    x = np.ascontiguousarray(inputs["x"], dtype=np.float32)
    in_maps = []
    for c in range(NCORES):
        b, half = c // 2, c % 2
        xc = np.ascontiguousarray(x[b, half * RPC:(half + 1) * RPC, :].T)
        in_maps.append({**shared, "xT": xc})
    res = run_bass_kernel_spmd(nc, in_maps, core_ids=list(range(NCORES)))
    outs = res.results
    y = np.empty((B, S, D), dtype=np.float32)
    Wu = inputs["Wu"].astype(np.float32)
    powers = (np.float32(g_bf) ** np.arange(1, RPC + 1, dtype=np.float32))
    for c in range(NCORES):
        b, half = c // 2, c % 2
        yc = outs[c]["y"]
        if half == 1:
            carry_vec = np.asarray(outs[c - 1]["carry"]).astype(
                np.float32).T.ravel()
            corr_row = carry_vec @ Wu
            yc = yc + powers[:, None] * corr_row[None, :]
        y[b, half * RPC:(half + 1) * RPC, :] = yc
    return y


# revision 16
# speedup vs baseline: 1.1817x; 1.1817x over previous
import sys

sys.path.insert(0, "/opt/trn_rl_repo")

import numpy as np
import ml_dtypes
from contextlib import ExitStack

import concourse.bass as bass
import concourse.bacc as bacc
import concourse.mybir as mybir
import concourse.tile as tile
from concourse.bass_utils import run_bass_kernel_spmd

B, S, D, MD = 4, 4096, 1024, 512
NCORES = 8
RPC = B * S // NCORES      # rows (tokens) per core = 2048
TT = 512                   # tokens per tile
NT = RPC // TT             # 4 tiles per core
P = 128
DB = D // P                # 8 k-blocks for D
MB = MD // P               # 4 blocks for MD
F32 = mybir.dt.float32
BF16 = mybir.dt.bfloat16
AF = mybir.ActivationFunctionType
OP = mybir.AluOpType
BF = ml_dtypes.bfloat16

_cache = {}


def _build_nc():
    nc = bacc.Bacc("TRN2", target_bir_lowering=False, debug=False,
                   num_devices=NCORES)

    xT = nc.dram_tensor("xT", [D, RPC], F32, kind="ExternalInput")
    wd = nc.dram_tensor("wd", [D, MD], BF16, kind="ExternalInput")
    wq = nc.dram_tensor("wq", [MD, MD], BF16, kind="ExternalInput")
    wk = nc.dram_tensor("wk", [MD, MD], BF16, kind="ExternalInput")
    wv = nc.dram_tensor("wv", [MD, MD], BF16, kind="ExternalInput")
    w0q = nc.dram_tensor("w0q", [MD, MD], BF16, kind="ExternalInput")
    w0k = nc.dram_tensor("w0k", [MD, MD], BF16, kind="ExternalInput")
    w1 = nc.dram_tensor("w1", [MD, MD], BF16, kind="ExternalInput")
    wu = nc.dram_tensor("wu", [MD, D], BF16, kind="ExternalInput")
    # per-partition biases [128, MB] fp32 (applied via ACT Identity)
    bd_i = nc.dram_tensor("bd_i", [P, MB], F32, kind="ExternalInput")
    bq_i = nc.dram_tensor("bq_i", [P, MB], F32, kind="ExternalInput")
    bk_i = nc.dram_tensor("bk_i", [P, MB], F32, kind="ExternalInput")
    bv_i = nc.dram_tensor("bv_i", [P, MB], F32, kind="ExternalInput")
    bu_row = nc.dram_tensor("bu_row", [1, D], BF16, kind="ExternalInput")
    # gelu biases stay per-partition (free via ACT)
    c0q_i = nc.dram_tensor("c0q_i", [P, MB], F32, kind="ExternalInput")
    c0k_i = nc.dram_tensor("c0k_i", [P, MB], F32, kind="ExternalInput")
    g_tile_i = nc.dram_tensor("g_tile_i", [P, RPC], BF16, kind="ExternalInput")
    lr_i = nc.dram_tensor("lr_i", [P, 1], F32, kind="ExternalInput")

    y = nc.dram_tensor("y", [RPC, D], F32, kind="ExternalOutput")
    carry = nc.dram_tensor("carry", [P, MB], BF16, kind="ExternalOutput")

    with ExitStack() as ctx:
        tc = ctx.enter_context(tile.TileContext(nc))
        wpool = ctx.enter_context(tc.tile_pool(name="wpool", bufs=1))
        persist = ctx.enter_context(tc.tile_pool(name="persist", bufs=1))
        work = ctx.enter_context(tc.tile_pool(name="work", bufs=2))
        stats = ctx.enter_context(tc.tile_pool(name="stats", bufs=3))
        psum = ctx.enter_context(tc.tile_pool(name="psum", bufs=4,
                                              space="PSUM"))
        psum2 = ctx.enter_context(tc.tile_pool(name="psum2", bufs=2,
                                               space="PSUM"))
        outp = ctx.enter_context(tc.tile_pool(name="outp", bufs=3))

        # ---- load weights / constants (once) ----
        wd_sb = wpool.tile([P, DB, MD], BF16)
        nc.sync.dma_start(wd_sb, wd.rearrange("(ko ki) m -> ki ko m", ki=P))
        w_sbs = {}
        for name, t in (("wq", wq), ("wk", wk), ("wv", wv), ("w0q", w0q),
                        ("w0k", w0k), ("w1", w1)):
            sb = wpool.tile([P, MB, MD], BF16, tag=name)
            nc.sync.dma_start(sb, t.rearrange("(ko ki) m -> ki ko m", ki=P))
            w_sbs[name] = sb
        wu_sb = wpool.tile([P, MB, D], BF16)
        nc.sync.dma_start(wu_sb, wu.rearrange("(ko ki) m -> ki ko m", ki=P))

        bias_sbs = {}
        for name, t in (("bd", bd_i), ("bq", bq_i), ("bk", bk_i),
                        ("bv", bv_i)):
            sb = wpool.tile([P, MB], F32, tag="b" + name)
            nc.sync.dma_start(sb, t[:])
            bias_sbs[name] = sb
        bu_sb = wpool.tile([1, D], BF16)
        nc.sync.dma_start(bu_sb, bu_row[:])
        c0_sbs = {}
        for name, t in (("c0q", c0q_i), ("c0k", c0k_i)):
            sb = wpool.tile([P, MB], F32, tag=name)
            nc.sync.dma_start(sb, t[:])
            c0_sbs[name] = sb
        g_tile = persist.tile([P, RPC], BF16)
        nc.sync.dma_start(g_tile, g_tile_i[:])
        lr_sb = wpool.tile([P, 1], F32)
        nc.sync.dma_start(lr_sb, lr_i[:])

        ones_mean = wpool.tile([P, P], BF16)
        nc.vector.memset(ones_mean, 1.0 / MD)
        ones_one = wpool.tile([P, P], BF16)
        nc.vector.memset(ones_one, 1.0)
        ones_row = wpool.tile([1, P], BF16)
        nc.vector.memset(ones_row, 1.0)
        eps_sb = wpool.tile([P, 1], F32)
        nc.vector.memset(eps_sb, 1e-5)

        scan_b = persist.tile([P, MB, RPC], BF16)

        def proj(h_bf, w_sb, bias_sb, tag, kblocks=MB):
            o = work.tile([P, MB, TT], BF16, tag=tag)
            for mb in range(MB):
                ps = psum.tile([P, TT], F32, tag="ps")
                for kb in range(kblocks):
                    nc.tensor.matmul(ps, w_sb[:, kb, mb * P:(mb + 1) * P],
                                     h_bf[:, kb, :], start=(kb == 0),
                                     stop=(kb == kblocks - 1))
                nc.scalar.activation(o[:, mb, :], ps, AF.Identity,
                                     bias=bias_sb[:, mb:mb + 1])
            return o

        def lnorm(pre, tag):
            sq = work.tile([P, MB, TT], BF16, tag="lnsq")
            nc.vector.tensor_mul(sq, pre, pre)
            m_ps = psum.tile([P, TT], F32, tag="ps")
            for kb in range(MB):
                nc.tensor.matmul(m_ps, ones_mean, pre[:, kb, :],
                                 start=(kb == 0), stop=(kb == MB - 1))
            e2_ps = psum.tile([P, TT], F32, tag="ps")
            for kb in range(MB):
                nc.tensor.matmul(e2_ps, ones_mean, sq[:, kb, :],
                                 start=(kb == 0), stop=(kb == MB - 1))
            m_sb = stats.tile([P, TT], BF16, tag="m")
            nc.vector.tensor_copy(m_sb, m_ps)
            msq = stats.tile([P, TT], BF16, tag="msq")
            nc.vector.tensor_mul(msq, m_sb, m_sb)
            var = stats.tile([P, TT], BF16, tag="var")
            nc.vector.tensor_tensor(var, e2_ps, msq, OP.subtract)
            std = stats.tile([P, TT], F32, tag="std")
            nc.scalar.activation(std, var, AF.Sqrt, bias=eps_sb)
            rstd_f = stats.tile([P, TT], F32, tag="rstdf")
            nc.vector.reciprocal_approx_fast(rstd_f, std)
            rstd = stats.tile([P, TT], BF16, tag="rstd")
            nc.vector.tensor_copy(rstd, rstd_f)
            mr = stats.tile([P, TT], BF16, tag="mr")
            nc.vector.tensor_mul(mr, m_sb, rstd)
            out_n = work.tile([P, MB, TT], BF16, tag="lnn")
            for kb in range(MB):
                nc.vector.tensor_mul(out_n[:, kb, :], pre[:, kb, :], rstd)
                nc.vector.tensor_tensor(out_n[:, kb, :], out_n[:, kb, :], mr,
                                        OP.subtract)
            return out_n

        def memmlp1(a_bf, w0name, c0name):
            g = work.tile([P, MB, TT], BF16, tag="gg")
            w0_sb = w_sbs[w0name]
            for mb in range(MB):
                ps = psum.tile([P, TT], F32, tag="ps")
                for kb in range(MB):
                    nc.tensor.matmul(ps, w0_sb[:, kb, mb * P:(mb + 1) * P],
                                     a_bf[:, kb, :], start=(kb == 0),
                                     stop=(kb == MB - 1))
                nc.scalar.activation(g[:, mb, :], ps, AF.Gelu_apprx_tanh,
                                     bias=c0_sbs[c0name][:, mb:mb + 1])
            return g

        def emit_out(t0, rs, off=0, ln=TT):
            for tb in range(ln // P):
                pso = psum2.tile([P, D], F32, tag="pso")
                for nh in range(2):
                    sl = slice(nh * 512, (nh + 1) * 512)
                    for kb in range(MB):
                        nc.tensor.matmul(
                            pso[:, sl],
                            rs[:, kb, off + tb * P:off + (tb + 1) * P],
                            wu_sb[:, kb, sl], start=(kb == 0), stop=False)
                    nc.tensor.matmul(pso[:, sl], ones_row, bu_sb[:, sl],
                                     start=False, stop=True)
                o_sb = outp.tile([P, D], F32, tag="osb")
                nc.scalar.activation(o_sb, pso, AF.Identity)
                nc.sync.dma_start(y[t0 + tb * P:t0 + (tb + 1) * P, :], o_sb)

        pending = None
        for ti in range(NT):
            t0 = ti * TT
            # load x with DMA-cast fp32->bf16 (software DGE casts in flight)
            xbf = work.tile([P, DB, TT], BF16, tag="xbf")
            for kb in range(DB):
                nc.gpsimd.dma_start(xbf[:, kb, :],
                                    xT[kb * P:(kb + 1) * P, t0:t0 + TT])
            # h = x@Wd + bd   (feature-major [MD, TT])
            h_bf = work.tile([P, MB, TT], BF16, tag="h")
            for mb in range(MB):
                ps = psum.tile([P, TT], F32, tag="ps")
                for kb in range(DB):
                    nc.tensor.matmul(ps, wd_sb[:, kb, mb * P:(mb + 1) * P],
                                     xbf[:, kb, :], start=(kb == 0),
                                     stop=(kb == DB - 1))
                nc.scalar.activation(h_bf[:, mb, :], ps, AF.Identity,
                                     bias=bias_sbs["bd"][:, mb:mb + 1])
            qpre = proj(h_bf, w_sbs["wq"], bias_sbs["bq"], "pre")
            kpre = proj(h_bf, w_sbs["wk"], bias_sbs["bk"], "pre")
            v_bf = proj(h_bf, w_sbs["wv"], bias_sbs["bv"], "vbf")
            qn = lnorm(qpre, "q")
            kn = lnorm(kpre, "k")
            if pending is not None:
                emit_out(*pending)
                pending = None
            gq = memmlp1(qn, "w0q", "c0q")
            gk = memmlp1(kn, "w0k", "c0k")
            # retrieved = gq @ W1
            retr = work.tile([P, MB, TT], BF16, tag="retr")
            w1_sb = w_sbs["w1"]
            for mb in range(MB):
                ps = psum.tile([P, TT], F32, tag="ps")
                for kb in range(MB):
                    nc.tensor.matmul(ps, w1_sb[:, kb, mb * P:(mb + 1) * P],
                                     gq[:, kb, :], start=(kb == 0),
                                     stop=(kb == MB - 1))
                nc.scalar.activation(retr[:, mb, :], ps, AF.Identity)
            # pred = gk @ W1 ; d = pred - v, then d^2 in place
            d_bf = work.tile([P, MB, TT], BF16, tag="d")
            for mb in range(MB):
                ps = psum.tile([P, TT], F32, tag="ps")
                for kb in range(MB):
                    nc.tensor.matmul(ps, w1_sb[:, kb, mb * P:(mb + 1) * P],
                                     gk[:, kb, :], start=(kb == 0),
                                     stop=(kb == MB - 1))
                nc.vector.tensor_tensor(d_bf[:, mb, :], ps, v_bf[:, mb, :],
                                        OP.subtract)
            nc.vector.tensor_mul(d_bf, d_bf, d_bf)
            ssum_ps = psum.tile([P, TT], F32, tag="ps")
            for kb in range(MB):
                nc.tensor.matmul(ssum_ps, ones_one, d_bf[:, kb, :],
                                 start=(kb == 0), stop=(kb == MB - 1))
            ssc = stats.tile([P, TT], BF16, tag="ssc")
            nc.vector.tensor_scalar_mul(ssc, ssum_ps, lr_sb)
            inp = work.tile([P, MB, TT], BF16, tag="inp")
            for mb in range(MB):
                nc.vector.tensor_mul(inp[:, mb, :], v_bf[:, mb, :], ssc)
            # chained scan along tokens; split the last tile's tail so the
            # final out-phase starts as soon as the first half has scanned
            halves = ((0, TT),) if ti < NT - 1 else ((0, TT // 2),
                                                     (TT // 2, TT))
            for (a, b) in halves:
                for mb in range(MB):
                    init = 0.0 if ti == 0 and a == 0 else                         scan_b[:, mb, t0 + a - 1:t0 + a]
                    nc.vector.tensor_tensor_scan(
                        scan_b[:, mb, t0 + a:t0 + b],
                        g_tile[:, t0 + a:t0 + b],
                        inp[:, mb, a:b], init, OP.mult, OP.add)
                nc.vector.tensor_tensor(retr[:, :, a:b], retr[:, :, a:b],
                                        scan_b[:, :, t0 + a:t0 + b], OP.add)
                if ti == NT - 1:
                    emit_out(t0 + a, retr, a, b - a)
            if ti < NT - 1:
                pending = (t0, retr)
        nc.sync.dma_start(carry[:], scan_b[:, :, RPC - 1])
    nc.compile()
    return nc


def _prep_shared(inputs):
    bf = lambda a: np.ascontiguousarray(a).astype(BF)
    f32 = lambda a: np.ascontiguousarray(a, dtype=np.float32)
    W0 = inputs["W0"].astype(np.float32)
    g_val = 1.0 - 1.0 / (1.0 + np.exp(-float(inputs["forget_factor"][0])))
    g_bf = float(np.float32(g_val).astype(BF))
    lr = float(inputs["adaptive_lr"][0])
    per_part = lambda b: f32(b.reshape(MB, P).T)  # [512] -> [128, MB]
    shared = {
        "wd": bf(inputs["Wd"]), "wq": bf(inputs["Wq"]), "wk": bf(inputs["Wk"]),
        "wv": bf(inputs["Wv"]),
        "w0q": bf(inputs["q_gamma"][:, None] * W0),
        "w0k": bf(inputs["k_gamma"][:, None] * W0),
        "w1": bf(inputs["W1"]), "wu": bf(inputs["Wu"]),
        "bd_i": per_part(inputs["bd"]), "bq_i": per_part(inputs["bq"]),
        "bk_i": per_part(inputs["bk"]), "bv_i": per_part(inputs["bv"]),
        "bu_row": bf(inputs["bu"][None, :]),
        "c0q_i": per_part(inputs["q_beta"].astype(np.float32) @ W0),
        "c0k_i": per_part(inputs["k_beta"].astype(np.float32) @ W0),
        "g_tile_i": np.full((P, RPC), g_bf, dtype=BF),
        "lr_i": np.full((P, 1), lr / MD, dtype=np.float32),
    }
    return shared, g_bf


def kernel(**inputs):
    if "nc" not in _cache:
        _cache["nc"] = _build_nc()
    nc = _cache["nc"]
    shared, g_bf = _prep_shared(inputs)
    x = np.ascontiguousarray(inputs["x"], dtype=np.float32)
    in_maps = []
    for c in range(NCORES):
        b, half = c // 2, c % 2
        xc = np.ascontiguousarray(x[b, half * RPC:(half + 1) * RPC, :].T)
        in_maps.append({**shared, "xT": xc})
    res = run_bass_kernel_spmd(nc, in_maps, core_ids=list(range(NCORES)))
    outs = res.results
    y = np.empty((B, S, D), dtype=np.float32)
    Wu = inputs["Wu"].astype(np.float32)
    powers = (np.float32(g_bf) ** np.arange(1, RPC + 1, dtype=np.float32))
    for c in range(NCORES):
        b, half = c // 2, c % 2
        yc = outs[c]["y"]
        if half == 1:
            carry_vec = np.asarray(outs[c - 1]["carry"]).astype(
                np.float32).T.ravel()
            corr_row = carry_vec @ Wu
            yc = yc + powers[:, None] * corr_row[None, :]
        y[b, half * RPC:(half + 1) * RPC, :] = yc
    return y
